# revision 3
# baseline (speedup 1.0000x reference)
"""Trainium2 Bass kernel for nn_ChannelizedLinearCompression.

Computation (fp32 reference):
    h1      = relu(einsum('bcn,cnh->bch', x, W1) + b1)   # [B, C, H]
    h2      = relu(einsum('bch,chk->bck', h1, W2) + b2)  # [B, C, 10]
    scalars = einsum('bck,ck->bc', h2, W3) + b3          # [B, C]
    out     = relu(scalars @ Wf1 + bf1) @ Wf2 + bf2      # [B, 16]

Sharding: 2 batch groups x 4 channel groups over 8 cores. Each core gets
x^T[c_loc, N, b_loc] (host-transposed so every big DMA is contiguous) and
computes scalars^T[c_loc, b_loc] on device; the tiny final MLP (0.003% of
the FLOPs) runs on host.

Stage 1 (99.95% of FLOPs) runs in fp8e4 (E4M3) with MatmulPerfMode.DoubleRow:
each matmul contracts K=256 (two 128-row k-tiles packed along dim-1 of both
operands) and streams a 2x512 moving block at 0.5 PE cycles per output row —
2x the fp16 MAC rate. End-to-end quantization error measured at 7.4e-4 relmax
vs the 2e-2 gate (the fp16 path was 8.6e-6). W1 is host-scaled by 64 so its
~0.02-sigma weights use the normal e4m3 range; the 1/64 is folded into the
PSUM-evacuation activation scale.

Walrus dual-fp8 ISA restrictions (found empirically): the LDWEIGHTS k-pair
stride must be a multiple of 64 — so W1 is host-packed per k-pair as
[128, 2, 320] (H padded 286->320 with zeros) — and the PSUM destination must
start at partition 0 — so each 64-wide M-chunk accumulates in its own
[64, 512] PSUM bank (5 banks), the batch half-loop sits outside the k loop,
W1 stays SBUF-resident across both halves, and h1 lands in five 64-partition
tiles from which stage 2 contracts in 64-row chunks.
"""

import os
from contextlib import ExitStack

import numpy as np

import concourse.bass as bass
import concourse.tile as tile
from concourse import bacc, mybir
from concourse.bass_utils import run_bass_kernel_spmd
from concourse._compat import get_trn_type

# Problem shapes (hardcoded; kernel.py must be self-contained).
B, C, N = 2048, 12, 8192
H, MID = 286, 10
FINAL_HIDDEN, LOWDIM = 30, 16
BG, CG = 2, 4  # batch groups x channel groups = 8 cores
B_LOC, C_LOC = B // BG, C // CG
HP = 320          # H padded to 5 chunks of 64 (dual-fp8 ldweights stride)
NKP = N // 256    # stage-1 K pairs (DoubleRow contracts 256 rows/matmul)

F32 = mybir.dt.float32
F16 = mybir.dt.float16
FP8 = mybir.dt.float8e4
RELU = mybir.ActivationFunctionType.Relu
IDENT = mybir.ActivationFunctionType.Identity
DOUBLE_ROW = mybir.MatmulPerfMode.DoubleRow

W1_SCALE = 64.0  # host-side W1 premultiplier; divided back out at evacuation

LAST = {}  # introspection for test.py (exec_time_ns etc.); harness ignores


def build_nc(b_loc=B_LOC, c_loc=C_LOC, n=N):
    assert n % 256 == 0 and b_loc % 1024 == 0
    nj = b_loc // 512
    mch = [(m0, min(64, H - m0)) for m0 in range(0, H, 64)]
    nm = len(mch)  # 5

    nc = bacc.Bacc(get_trn_type() or "TRN2", target_bir_lowering=False)
    xt = nc.declare_dram_parameter("xt", [c_loc, n, b_loc], FP8, isOutput=False)
    w1 = nc.declare_dram_parameter("w1", [c_loc, NKP, 128, 2, HP], FP8,
                                   isOutput=False)
    b1 = nc.declare_dram_parameter("b1", [c_loc, H, 1], F32, isOutput=False)
    w2 = nc.declare_dram_parameter("w2", [c_loc, H, MID], F16, isOutput=False)
    b2 = nc.declare_dram_parameter("b2", [c_loc, MID, 1], F32, isOutput=False)
    w3 = nc.declare_dram_parameter("w3", [c_loc, MID, 1], F16, isOutput=False)
    b3 = nc.declare_dram_parameter("b3", [c_loc, 1, 1], F32, isOutput=False)
    out = nc.declare_dram_parameter("out", [c_loc, b_loc], F32, isOutput=True)

    with tile.TileContext(nc) as tc, ExitStack() as ctx:
        xp = ctx.enter_context(tc.tile_pool(name="xp", bufs=6))
        wp = ctx.enter_context(tc.tile_pool(name="wp", bufs=2 * NKP))
        hp = ctx.enter_context(tc.tile_pool(name="hp", bufs=2 * nm))
        sp = ctx.enter_context(tc.tile_pool(name="sp", bufs=30))
        op = ctx.enter_context(tc.tile_pool(name="op", bufs=4))
        pp = ctx.enter_context(
            tc.tile_pool(name="pp", bufs=8, space=bass.MemorySpace.PSUM)
        )

        for c in range(c_loc):
            b1t = [sp.tile([64, 1], F32, tag="b1t", name=f"b1t{c}_{i}")
                   for i in range(nm)]
            w2t = [sp.tile([64, MID], F16, tag="w2t", name=f"w2t{c}_{i}")
                   for i in range(nm)]
            for i, (m0, ms) in enumerate(mch):
                nc.sync.dma_start(b1t[i][:ms, :], b1[c, m0:m0 + ms, :])
                nc.sync.dma_start(w2t[i][:ms, :], w2[c, m0:m0 + ms, :])
            w3t = sp.tile([MID, 1], F16, tag="w3t", name=f"w3t{c}")
            b2t = sp.tile([MID, 1], F32, tag="b2t", name=f"b2t{c}")
            b3t = sp.tile([1, 1], F32, tag="b3t", name=f"b3t{c}")
            nc.sync.dma_start(w3t[:, :], w3[c])
            nc.sync.dma_start(b2t[:, :], b2[c])
            nc.sync.dma_start(b3t[:, :], b3[c])

            # stage 1: h1T[h, b] = relu((W1[c].T @ xT[c]) / 64 + b1[c]),
            # fp8 DoubleRow; W1 SBUF-resident, batch halves outer
            w1ts = [wp.tile([128, 2, HP], FP8, tag="w1t", name=f"w1t{c}_{kp}")
                    for kp in range(NKP)]
            h1m = [hp.tile([64, b_loc], F16, tag="h1m", name=f"h1m{c}_{i}")
                   for i in range(nm)]
            for jn in range(nj):
                j0 = jn * 512
                ps = [pp.tile([64, 512], F32, tag="ps", name=f"ps{c}_{jn}_{i}")
                      for i in range(nm)]
                for kp in range(NKP):
                    k0 = kp * 256
                    if jn == 0:
                        nc.sync.dma_start(w1ts[kp][:, :, :], w1[c, kp])
                    xtt = xp.tile([128, 2, 512], FP8, tag="xtt",
                                  name=f"xtt{c}_{jn}_{kp}")
                    nc.sync.dma_start(xtt[:, 0:1, :],
                                      xt[c, k0:k0 + 128, j0:j0 + 512])
                    nc.sync.dma_start(xtt[:, 1:2, :],
                                      xt[c, k0 + 128:k0 + 256, j0:j0 + 512])
                    for mi in range(nm):
                        nc.tensor.matmul(
                            ps[mi][:, :],
                            w1ts[kp][:, :, mi * 64:(mi + 1) * 64],
                            xtt[:, :, :],
                            start=(kp == 0),
                            stop=(kp == NKP - 1),
                            perf_mode=DOUBLE_ROW,
                        )
                for mi, (m0, ms) in enumerate(mch):
                    nc.scalar.activation(
                        h1m[mi][:ms, j0:j0 + 512],
                        ps[mi][:ms, :],
                        RELU,
                        bias=b1t[mi][:ms, :],
                        scale=1.0 / W1_SCALE,
                    )

            # stage 2: h2T[k, b] = relu(W2[c].T @ h1T + b2[c]), 64-row chunks
            p2 = [pp.tile([MID, 512], F32, tag="ps", name=f"p2{c}_{j}")
                  for j in range(nj)]
            for mi, (m0, ms) in enumerate(mch):
                for j in range(nj):
                    nc.tensor.matmul(
                        p2[j][:, :],
                        w2t[mi][:ms, :],
                        h1m[mi][:ms, j * 512:(j + 1) * 512],
                        start=(mi == 0),
                        stop=(mi == nm - 1),
                    )
            h2t = op.tile([MID, b_loc], F16, tag="h2t", name=f"h2t{c}")
            for j in range(nj):
                nc.scalar.activation(
                    h2t[:, j * 512:(j + 1) * 512], p2[j][:, :], RELU,
                    bias=b2t[:, :],
                )

            # stage 3: scalarsT[c, b] = W3[c].T @ h2T + b3[c]
            p3 = [pp.tile([1, 512], F32, tag="ps", name=f"p3{c}_{j}")
                  for j in range(nj)]
            sct = op.tile([1, b_loc], F32, tag="sct", name=f"sct{c}")
            for j in range(nj):
                nc.tensor.matmul(
                    p3[j][:, :], w3t[:, :], h2t[:, j * 512:(j + 1) * 512],
                    start=True, stop=True,
                )
                nc.scalar.activation(
                    sct[:, j * 512:(j + 1) * 512], p3[j][:, :], IDENT,
                    bias=b3t[:, :],
                )
            nc.sync.dma_start(out[c:c + 1, :], sct[0:1, :])

    nc.compile()
    return nc


_NC_CACHE = {}


def _get_nc():
    key = (B_LOC, C_LOC, N)
    if key not in _NC_CACHE:
        _NC_CACHE[key] = build_nc()
    return _NC_CACHE[key]


def _to_fp8(arr, scale=1.0):
    import ml_dtypes
    a = np.asarray(arr, np.float32)
    if scale != 1.0:
        a = a * scale
    return np.ascontiguousarray(a).astype(ml_dtypes.float8_e4m3)


def _to_f16(arr):
    return np.ascontiguousarray(arr, dtype=np.float16)


def _pack_w1(W1c):
    """[c_loc, n, H] f32 -> [c_loc, NKP, 128, 2, HP] fp8, scaled by 64."""
    import ml_dtypes
    c_loc = W1c.shape[0]
    w = np.zeros((c_loc, N, HP), dtype=ml_dtypes.float8_e4m3)
    w[:, :, :H] = _to_fp8(W1c, W1_SCALE)
    # n -> (kp, i, p) with p the partition (inner 128) and i the pair index
    w = w.reshape(c_loc, NKP, 2, 128, HP).transpose(0, 1, 3, 2, 4)
    return np.ascontiguousarray(w)


def _transpose_shard(xs):
    """[b_loc, c_loc, n] -> contiguous [c_loc, n, b_loc]."""
    try:
        import torch
        try:
            torch.set_num_threads(max(os.cpu_count() or 1, 1))
        except Exception:
            pass
        return torch.from_numpy(
            np.ascontiguousarray(xs).view(np.uint8)
        ).permute(1, 2, 0).contiguous().numpy()
    except ImportError:
        return np.ascontiguousarray(np.transpose(xs, (1, 2, 0)))


def kernel(x, W1, b1, W2, b2, W3, b3, Wf1, bf1, Wf2, bf2):
    import ml_dtypes

    x = np.asarray(x, dtype=np.float32)
    W1 = np.asarray(W1, dtype=np.float32)
    b1 = np.asarray(b1, dtype=np.float32)
    W2 = np.asarray(W2, dtype=np.float32)
    b2 = np.asarray(b2, dtype=np.float32)
    W3 = np.asarray(W3, dtype=np.float32)
    b3 = np.asarray(b3, dtype=np.float32)

    nc = _get_nc()

    # cast to 1-byte fp8 before transposing so the shuffle moves 1/4 the bytes
    x8 = _to_fp8(x)

    # per-channel-group tensors are identical across batch groups; build once
    cg_maps = []
    for ic in range(CG):
        cs = slice(ic * C_LOC, (ic + 1) * C_LOC)
        cg_maps.append({
            "w1": _pack_w1(W1[cs]),
            "b1": np.ascontiguousarray(b1[cs])[:, :, None],
            "w2": _to_f16(W2[cs]),
            "b2": np.ascontiguousarray(b2[cs])[:, :, None],
            "w3": _to_f16(W3[cs])[:, :, None],
            "b3": np.ascontiguousarray(b3[cs])[:, None, None],
        })

    in_maps = []
    for ib in range(BG):
        bs = slice(ib * B_LOC, (ib + 1) * B_LOC)
        for ic in range(CG):
            cs = slice(ic * C_LOC, (ic + 1) * C_LOC)
            m = dict(cg_maps[ic])
            m["xt"] = _transpose_shard(x8[bs, cs, :]).view(
                ml_dtypes.float8_e4m3)
            in_maps.append(m)

    res = run_bass_kernel_spmd(nc, in_maps, list(range(BG * CG)))
    LAST["exec_time_ns"] = res.exec_time_ns
    LAST["results"] = res

    scalars = np.empty((B, C), np.float32)
    idx = 0
    for ib in range(BG):
        bs = slice(ib * B_LOC, (ib + 1) * B_LOC)
        for ic in range(CG):
            cs = slice(ic * C_LOC, (ic + 1) * C_LOC)
            scalars[bs, cs] = res.results[idx]["out"].T
            idx += 1

    # Final tiny MLP (C -> 30 -> lowdim) on host in fp32.
    h = np.maximum(scalars @ np.asarray(Wf1, np.float32)
                   + np.asarray(bf1, np.float32), 0.0)
    return (h @ np.asarray(Wf2, np.float32)
            + np.asarray(bf2, np.float32)).astype(np.float32)


# revision 4
# speedup vs baseline: 1.5899x; 1.5899x over previous
"""Trainium2 Bass kernel for nn_ChannelizedLinearCompression.

Computation (fp32 reference):
    h1      = relu(einsum('bcn,cnh->bch', x, W1) + b1)   # [B, C, H]
    h2      = relu(einsum('bch,chk->bck', h1, W2) + b2)  # [B, C, 10]
    scalars = einsum('bck,ck->bc', h2, W3) + b3          # [B, C]
    out     = relu(scalars @ Wf1 + bf1) @ Wf2 + bf2      # [B, 16]

Sharding: 2 batch groups x 4 channel groups over 8 cores. Each core gets
x^T[c_loc, N, b_loc] (host-transposed so every big DMA is contiguous) and
computes scalars^T[c_loc, b_loc] on device; the tiny final MLP (0.003% of
the FLOPs) runs on host.

Stage 1 (99.95% of FLOPs) runs in fp8e4 (E4M3) with MatmulPerfMode.DoubleRow
at M=128: each matmul holds a 2x128 stationary (two fp8 weights packed per PE
cell, one per 128-row k-tile) and streams 2x512 moving bytes at 2B/cycle/
partition — K=256 x M=128 = 32768 MACs/cycle, 2x the fp16 peak. End-to-end
quantization error measured at 7.4e-4 relmax vs the 2e-2 gate (fp16 path was
8.6e-6). W1 is host-scaled by 64 so its ~0.02-sigma weights use the normal
e4m3 range; the 1/64 is folded into the PSUM-evacuation activation scale.

Walrus dual-fp8 ISA restrictions (found empirically): the LDWEIGHTS k-pair
stride must be a multiple of 64 — so W1 is host-packed per 256-row k-pair as
[128, 2, 320] (H padded 286->320 with zeros, giving M-chunks 128/128/64) —
and the PSUM destination must start at partition 0 (all chunks do).

Device per-core dataflow (per local channel c):
  stage1: for each K pair (32 of N=8192): psum[m_chunk][b_half] +=
          W1[kp,:,m_chunk].T @ xT[kp,:,b_half]  (DoubleRow fp8; h1 lands
          h-major so stages 2/3 chain with no transposes; b1 + 1/64 scale
          applied by ScalarE at evacuation)
  stage2: psum2[b_half] += W2[h_chunk].T @ h1T[h_chunk, b_half]; relu+b2
  stage3: psum3[b_half] = W3.T @ h2T[:, b_half]; +b3 -> scalars^T row
"""

import os
from contextlib import ExitStack

import numpy as np

import concourse.bass as bass
import concourse.tile as tile
from concourse import bacc, mybir
from concourse.bass_utils import run_bass_kernel_spmd
from concourse._compat import get_trn_type

# Problem shapes (hardcoded; kernel.py must be self-contained).
B, C, N = 2048, 12, 8192
H, MID = 286, 10
FINAL_HIDDEN, LOWDIM = 30, 16
BG, CG = 2, 4  # batch groups x channel groups = 8 cores
B_LOC, C_LOC = B // BG, C // CG
HP = 320          # H padded (zeros) so dual-fp8 ldweights strides are 64k
NKP = N // 256    # stage-1 K pairs (DoubleRow contracts 256 rows/matmul)
# stage-1 M chunks: (offset, stationary cols, valid h rows)
MCH = [(0, 128, 128), (128, 128, 128), (256, 64, 30)]

F32 = mybir.dt.float32
F16 = mybir.dt.float16
FP8 = mybir.dt.float8e4
RELU = mybir.ActivationFunctionType.Relu
IDENT = mybir.ActivationFunctionType.Identity
DOUBLE_ROW = mybir.MatmulPerfMode.DoubleRow

W1_SCALE = 64.0  # host-side W1 premultiplier; divided back out at evacuation

LAST = {}  # introspection for test.py (exec_time_ns etc.); harness ignores


def build_nc(b_loc=B_LOC, c_loc=C_LOC, n=N):
    assert n % 256 == 0 and b_loc % 1024 == 0
    nj = b_loc // 512

    nc = bacc.Bacc(get_trn_type() or "TRN2", target_bir_lowering=False)
    xt = nc.declare_dram_parameter("xt", [c_loc, n, b_loc], FP8, isOutput=False)
    w1 = nc.declare_dram_parameter("w1", [c_loc, NKP, 128, 2, HP], FP8,
                                   isOutput=False)
    b1 = nc.declare_dram_parameter("b1", [c_loc, H, 1], F32, isOutput=False)
    w2 = nc.declare_dram_parameter("w2", [c_loc, H, MID], F16, isOutput=False)
    b2 = nc.declare_dram_parameter("b2", [c_loc, MID, 1], F32, isOutput=False)
    w3 = nc.declare_dram_parameter("w3", [c_loc, MID, 1], F16, isOutput=False)
    b3 = nc.declare_dram_parameter("b3", [c_loc, 1, 1], F32, isOutput=False)
    out = nc.declare_dram_parameter("out", [c_loc, b_loc], F32, isOutput=True)

    with tile.TileContext(nc) as tc, ExitStack() as ctx:
        xp = ctx.enter_context(tc.tile_pool(name="xp", bufs=6))
        wp = ctx.enter_context(tc.tile_pool(name="wp", bufs=6))
        hp = ctx.enter_context(tc.tile_pool(name="hp", bufs=2 * len(MCH)))
        sp = ctx.enter_context(tc.tile_pool(name="sp", bufs=30))
        op = ctx.enter_context(tc.tile_pool(name="op", bufs=4))
        pp = ctx.enter_context(
            tc.tile_pool(name="pp", bufs=8, space=bass.MemorySpace.PSUM)
        )

        for c in range(c_loc):
            b1t = [sp.tile([hs, 1], F32, tag=f"b1t{i}", name=f"b1t{c}_{i}")
                   for i, (m0, ms, hs) in enumerate(MCH)]
            w2t = [sp.tile([hs, MID], F16, tag=f"w2t{i}", name=f"w2t{c}_{i}")
                   for i, (m0, ms, hs) in enumerate(MCH)]
            for i, (m0, ms, hs) in enumerate(MCH):
                nc.sync.dma_start(b1t[i][:hs, :], b1[c, m0:m0 + hs, :])
                nc.sync.dma_start(w2t[i][:hs, :], w2[c, m0:m0 + hs, :])
            w3t = sp.tile([MID, 1], F16, tag="w3t", name=f"w3t{c}")
            b2t = sp.tile([MID, 1], F32, tag="b2t", name=f"b2t{c}")
            b3t = sp.tile([1, 1], F32, tag="b3t", name=f"b3t{c}")
            nc.sync.dma_start(w3t[:, :], w3[c])
            nc.sync.dma_start(b2t[:, :], b2[c])
            nc.sync.dma_start(b3t[:, :], b3[c])

            # stage 1: h1T[h, b] = relu((W1[c].T @ xT[c]) / 64 + b1[c])
            ps = [[pp.tile([ms, 512], F32, tag="ps", name=f"ps{c}_{i}_{j}")
                   for j in range(nj)] for i, (m0, ms, hs) in enumerate(MCH)]
            for kp in range(NKP):
                k0 = kp * 256
                xtt = xp.tile([128, 2, b_loc], FP8, tag="xtt",
                              name=f"xtt{c}_{kp}")
                w1t = wp.tile([128, 2, HP], FP8, tag="w1t", name=f"w1t{c}_{kp}")
                nc.sync.dma_start(xtt[:, 0:1, :], xt[c, k0:k0 + 128, :])
                nc.sync.dma_start(xtt[:, 1:2, :], xt[c, k0 + 128:k0 + 256, :])
                nc.sync.dma_start(w1t[:, :, :], w1[c, kp])
                for i, (m0, ms, hs) in enumerate(MCH):
                    for j in range(nj):
                        nc.tensor.matmul(
                            ps[i][j][:ms, :],
                            w1t[:, :, m0:m0 + ms],
                            xtt[:, :, j * 512:(j + 1) * 512],
                            start=(kp == 0),
                            stop=(kp == NKP - 1),
                            perf_mode=DOUBLE_ROW,
                        )
            h1t = [hp.tile([hs, b_loc], F16, tag=f"h1t{i}",
                           name=f"h1t{c}_{i}")
                   for i, (m0, ms, hs) in enumerate(MCH)]
            for i, (m0, ms, hs) in enumerate(MCH):
                for j in range(nj):
                    nc.scalar.activation(
                        h1t[i][:hs, j * 512:(j + 1) * 512],
                        ps[i][j][:hs, :],
                        RELU,
                        bias=b1t[i][:hs, :],
                        scale=1.0 / W1_SCALE,
                    )

            # stage 2: h2T[k, b] = relu(W2[c].T @ h1T + b2[c])
            p2 = [pp.tile([MID, 512], F32, tag="ps", name=f"p2{c}_{j}")
                  for j in range(nj)]
            for i, (m0, ms, hs) in enumerate(MCH):
                for j in range(nj):
                    nc.tensor.matmul(
                        p2[j][:, :],
                        w2t[i][:hs, :],
                        h1t[i][:hs, j * 512:(j + 1) * 512],
                        start=(i == 0),
                        stop=(i == len(MCH) - 1),
                    )
            h2t = op.tile([MID, b_loc], F16, tag="h2t", name=f"h2t{c}")
            for j in range(nj):
                nc.scalar.activation(
                    h2t[:, j * 512:(j + 1) * 512], p2[j][:, :], RELU,
                    bias=b2t[:, :],
                )

            # stage 3: scalarsT[c, b] = W3[c].T @ h2T + b3[c]
            p3 = [pp.tile([1, 512], F32, tag="ps", name=f"p3{c}_{j}")
                  for j in range(nj)]
            sct = op.tile([1, b_loc], F32, tag="sct", name=f"sct{c}")
            for j in range(nj):
                nc.tensor.matmul(
                    p3[j][:, :], w3t[:, :], h2t[:, j * 512:(j + 1) * 512],
                    start=True, stop=True,
                )
                nc.scalar.activation(
                    sct[:, j * 512:(j + 1) * 512], p3[j][:, :], IDENT,
                    bias=b3t[:, :],
                )
            nc.sync.dma_start(out[c:c + 1, :], sct[0:1, :])

    nc.compile()
    return nc


_NC_CACHE = {}


def _get_nc():
    key = (B_LOC, C_LOC, N)
    if key not in _NC_CACHE:
        _NC_CACHE[key] = build_nc()
    return _NC_CACHE[key]


def _to_fp8(arr, scale=1.0):
    import ml_dtypes
    a = np.asarray(arr, np.float32)
    if scale != 1.0:
        a = a * scale
    return np.ascontiguousarray(a).astype(ml_dtypes.float8_e4m3)


def _to_f16(arr):
    return np.ascontiguousarray(arr, dtype=np.float16)


def _pack_w1(W1c):
    """[c_loc, n, H] f32 -> [c_loc, NKP, 128, 2, HP] fp8, scaled by 64."""
    import ml_dtypes
    c_loc = W1c.shape[0]
    w = np.zeros((c_loc, N, HP), dtype=ml_dtypes.float8_e4m3)
    w[:, :, :H] = _to_fp8(W1c, W1_SCALE)
    # n -> (kp, i, p) with p the partition (inner 128) and i the pair index
    w = w.reshape(c_loc, NKP, 2, 128, HP).transpose(0, 1, 3, 2, 4)
    return np.ascontiguousarray(w)


def _transpose_shard(xs):
    """[b_loc, c_loc, n] -> contiguous [c_loc, n, b_loc]."""
    try:
        import torch
        try:
            torch.set_num_threads(max(os.cpu_count() or 1, 1))
        except Exception:
            pass
        return torch.from_numpy(
            np.ascontiguousarray(xs).view(np.uint8)
        ).permute(1, 2, 0).contiguous().numpy()
    except ImportError:
        return np.ascontiguousarray(np.transpose(xs, (1, 2, 0)))


def kernel(x, W1, b1, W2, b2, W3, b3, Wf1, bf1, Wf2, bf2):
    import ml_dtypes

    x = np.asarray(x, dtype=np.float32)
    W1 = np.asarray(W1, dtype=np.float32)
    b1 = np.asarray(b1, dtype=np.float32)
    W2 = np.asarray(W2, dtype=np.float32)
    b2 = np.asarray(b2, dtype=np.float32)
    W3 = np.asarray(W3, dtype=np.float32)
    b3 = np.asarray(b3, dtype=np.float32)

    nc = _get_nc()

    # cast to 1-byte fp8 before transposing so the shuffle moves 1/4 the bytes
    x8 = _to_fp8(x)

    # per-channel-group tensors are identical across batch groups; build once
    cg_maps = []
    for ic in range(CG):
        cs = slice(ic * C_LOC, (ic + 1) * C_LOC)
        cg_maps.append({
            "w1": _pack_w1(W1[cs]),
            "b1": np.ascontiguousarray(b1[cs])[:, :, None],
            "w2": _to_f16(W2[cs]),
            "b2": np.ascontiguousarray(b2[cs])[:, :, None],
            "w3": _to_f16(W3[cs])[:, :, None],
            "b3": np.ascontiguousarray(b3[cs])[:, None, None],
        })

    in_maps = []
    for ib in range(BG):
        bs = slice(ib * B_LOC, (ib + 1) * B_LOC)
        for ic in range(CG):
            cs = slice(ic * C_LOC, (ic + 1) * C_LOC)
            m = dict(cg_maps[ic])
            m["xt"] = _transpose_shard(x8[bs, cs, :]).view(
                ml_dtypes.float8_e4m3)
            in_maps.append(m)

    res = run_bass_kernel_spmd(nc, in_maps, list(range(BG * CG)))
    LAST["exec_time_ns"] = res.exec_time_ns
    LAST["results"] = res

    scalars = np.empty((B, C), np.float32)
    idx = 0
    for ib in range(BG):
        bs = slice(ib * B_LOC, (ib + 1) * B_LOC)
        for ic in range(CG):
            cs = slice(ic * C_LOC, (ic + 1) * C_LOC)
            scalars[bs, cs] = res.results[idx]["out"].T
            idx += 1

    # Final tiny MLP (C -> 30 -> lowdim) on host in fp32.
    h = np.maximum(scalars @ np.asarray(Wf1, np.float32)
                   + np.asarray(bf1, np.float32), 0.0)
    return (h @ np.asarray(Wf2, np.float32)
            + np.asarray(bf2, np.float32)).astype(np.float32)
